# revision 22
# baseline (speedup 1.0000x reference)
"""Trainium2 Bass kernel for the CRW (contrastive random walk) module.

Shapes hardcoded for: seq (8,10,128,32,32) f32, conv1 1->32 s2 SAME, conv2
32->64 s2 SAME, GAP, linear 64->128, L2-normalize, consecutive-frame
affinities A (8,9,128,128) = emb_t . emb_{t+1}^T / 0.07, row softmax S over
[A, flip-transpose(A)], cycle-walk chain products + diag log_softmax loss.

Sharding: data-parallel over B=8, one batch element per NeuronCore. Host
sums the per-core loss partials and stacks the 9 affinity matrices.

Device mapping:
- conv1: one matmul per 4-image group, K=36 (4 img x 9 taps, host im2col),
  M=128 (4 img x 32 ch), block-diagonal weights. jax-SAME padding (0,1).
- conv2: 9 PSUM-accumulating matmuls (one per tap), K=64 (2 img x 32 ch),
  M=128 (2 img x 64 ch), strided APs over a zero-padded 18x18 h1 layout.
- chunks software-pipelined (conv1 of chunk c+1 emitted before conv2 of
  chunk c) so the in-order PE stream never head-blocks on the shuffle DMAs.
- pooling mean folded into linear weights; linear bias via a K=1
  accumulating matmul; embeddings normalized + transposed per frame as
  soon as their chunks finish, hiding the whole phase under conv compute.
- softmax over all 18 matrices and the 8 log-softmax loss reductions run
  as a few wide instructions; the 8 chain products interleave round-robin.
"""

import numpy as np

import concourse.bass as bass
import concourse.bacc as bacc
import concourse.mybir as mybir
from concourse.tile import TileContext
from concourse.bass_utils import run_bass_kernel_spmd

F32 = mybir.dt.float32
MMDT = mybir.dt.float32r  # matmul operand dtype

TAU = 0.07
B, T, N, H, W = 8, 10, 128, 32, 32
NIMG = T * N            # 1280 images per core
CHUNK = 64              # images per pipeline chunk
NCHUNK = NIMG // CHUNK  # 20
NPAIR = CHUNK // 2      # 32 pairs per chunk
PADZ = 18 * 18


def _ap(base, extra_off, dims):
    return bass.AP(tensor=base.tensor, offset=base.offset + extra_off, ap=dims)


def _build():
    nc = bacc.Bacc()

    rx = nc.dram_tensor("rx", [NCHUNK, 36, 16 * 256], MMDT, kind="ExternalInput")
    w1p = nc.dram_tensor("w1p", [36, 128], MMDT, kind="ExternalInput")
    w2p = nc.dram_tensor("w2p", [64, 9, 128], MMDT, kind="ExternalInput")
    lin = nc.dram_tensor("lin", [128, 128], MMDT, kind="ExternalInput")
    linb = nc.dram_tensor("linb", [128, 128], MMDT, kind="ExternalInput")
    c1b_d = nc.dram_tensor("c1b", [128, 1], F32, kind="ExternalInput")
    c2b_d = nc.dram_tensor("c2b", [128, 1], F32, kind="ExternalInput")
    ones_d = nc.dram_tensor("onesr", [128, 64], MMDT, kind="ExternalInput")
    identr_d = nc.dram_tensor("identr", [128, 128], MMDT, kind="ExternalInput")
    identf_d = nc.dram_tensor("identf_in", [128, 128], F32, kind="ExternalInput")
    a_out = nc.dram_tensor("A_out", [9, 128, 128], F32, kind="ExternalOutput")
    lossp = nc.dram_tensor("lossp", [128, 1], F32, kind="ExternalOutput")

    with TileContext(nc) as tc:
        with tc.tile_pool(name="consts", bufs=1) as consts, \
             tc.tile_pool(name="persist", bufs=1) as persist:
            w1s = consts.tile([36, 128], MMDT)
            nc.sync.dma_start(out=w1s, in_=w1p[:, :])
            w2s = consts.tile([64, 9, 128], MMDT)
            nc.scalar.dma_start(out=w2s, in_=w2p[:, :, :])
            lin_s = consts.tile([128, 128], MMDT)
            nc.sync.dma_start(out=lin_s, in_=lin[:, :])
            linb_s = consts.tile([128, 128], MMDT)
            nc.scalar.dma_start(out=linb_s, in_=linb[:, :])
            c1b = consts.tile([128, 1], F32)
            nc.sync.dma_start(out=c1b, in_=c1b_d[:, :])
            c2b = consts.tile([128, 1], F32)
            nc.sync.dma_start(out=c2b, in_=c2b_d[:, :])
            ones1 = consts.tile([128, 64], MMDT)
            nc.sync.dma_start(out=ones1, in_=ones_d[:, :])
            ident = consts.tile([128, 128], MMDT)
            nc.scalar.dma_start(out=ident, in_=identr_d[:, :])
            identf = consts.tile([128, 128], F32)
            nc.scalar.dma_start(out=identf, in_=identf_d[:, :])

            pooled_sb = persist.tile([128, 640], MMDT)
            embnT = persist.tile([128, NIMG], MMDT)
            tmps = [persist.tile([128, 16, PADZ], MMDT, tag=f"tmp{i}",
                                 name=f"tmp{i}")
                    for i in range(2)]
            for tmp in tmps:
                nc.gpsimd.memset(tmp[:, :, :].bitcast(F32), 0.0)

            # -------- conv encoder + per-frame embeddings, pipelined --------
            with tc.tile_pool(name="cv_sb", bufs=3) as cv_sb, \
                 tc.tile_pool(name="cv_h1", bufs=2) as cv_h1, \
                 tc.tile_pool(name="cv_t2", bufs=3) as cv_t2, \
                 tc.tile_pool(name="em_sb", bufs=2) as em_sb, \
                 tc.tile_pool(name="ps1", bufs=2, space="PSUM") as ps1, \
                 tc.tile_pool(name="ps2", bufs=2, space="PSUM") as ps2, \
                 tc.tile_pool(name="psE", bufs=1, space="PSUM") as psE_p, \
                 tc.tile_pool(name="psR", bufs=1, space="PSUM") as psR_p:

                def conv1_sub(c, rhs, h1c, ppitch, sub):
                    tmp = tmps[c % 2]
                    tap = list(tmp[:, :, :].ap[0])
                    for q in range(4):
                        p1 = ps1.tile([128, 512], F32, tag="p1", name="p1",
                                      bufs=4)
                        for j in range(2):
                            g = sub * 8 + q * 2 + j
                            nc.tensor.matmul(
                                p1[:, j * 256:(j + 1) * 256], w1s,
                                rhs[:, g * 256:(g + 1) * 256],
                                start=True, stop=True)
                        src = p1.rearrange("p (g y x) -> p g y x", g=2, y=16)
                        dst = _ap(tmp[:, :, :], (sub * 8 + q * 2) * PADZ,
                                  [tap, [PADZ, 2], [18, 16], [1, 16]])
                        if q % 2 == 0:
                            nc.scalar.activation(
                                out=dst, in_=src,
                                func=mybir.ActivationFunctionType.Relu,
                                bias=c1b)
                        else:
                            nc.vector.tensor_scalar(
                                out=dst, in0=src, scalar1=c1b, scalar2=0.0,
                                op0=mybir.AluOpType.add,
                                op1=mybir.AluOpType.max)
                    # shuffle this sub-half into conv2 layout
                    for sh in range(2):
                        dst = _ap(h1c[0:64, 0, :], (sub * 16 + sh) * PADZ,
                                  [[ppitch[0], 64], [2 * PADZ, 8], [1, PADZ]])
                        nc.sync.dma_start(
                            out=dst,
                            in_=tmp[sh * 64:(sh + 1) * 64,
                                    sub * 8:sub * 8 + 8, :])

                def conv2_half(c, h1c, hh):
                    ppitch = list(h1c[:, :, :].ap[0])
                    for h in (hh * 2, hh * 2 + 1):
                        p2 = ps2.tile([128, 512], F32, tag="p2", name="p2")
                        for t in range(9):
                            dy, dx = t // 3, t % 3
                            rhs2 = _ap(h1c[:, 0, :],
                                       h * 8 * PADZ + 18 * dy + dx,
                                       [[ppitch[0], 64], [PADZ, 8], [36, 8],
                                        [2, 8]])
                            nc.tensor.matmul(p2, w2s[:, t, :], rhs2,
                                             start=(t == 0), stop=(t == 8))
                        t2 = cv_t2.tile([128, 512], F32, tag=f"t2{h % 2}",
                                        name="t2")
                        nc.scalar.activation(
                            out=t2, in_=p2,
                            func=mybir.ActivationFunctionType.Relu, bias=c2b)
                        with nc.allow_low_precision(
                                reason="pooled f32r matmul operands"):
                            nc.vector.reduce_sum(
                                out=pooled_sb[:, c * 32 + h * 8:
                                              c * 32 + h * 8 + 8],
                                in_=t2.rearrange("p (i s) -> p i s", s=64),
                                axis=mybir.AxisListType.X)

                embns = {}

                def emb_mm_part(f):
                    # frame f: pairs f*64..(f+1)*64, parities on partition halves
                    psE = psE_p.tile([64, 256], F32, tag="psE", name="psE")
                    for sp in range(2):
                        sl = psE[:, sp * 128:(sp + 1) * 128]
                        b0 = sp * 64
                        nc.tensor.matmul(
                            sl, pooled_sb[b0:b0 + 64, f * 64:(f + 1) * 64],
                            lin_s[b0:b0 + 64, :], start=True, stop=False)
                        nc.tensor.matmul(sl, ones1[b0:b0 + 1, :],
                                         linb_s[b0:b0 + 1, :],
                                         start=False, stop=True)
                    emb_sb = em_sb.tile([64, 2, 128], F32, tag="emb", name="emb")
                    nc.scalar.copy(emb_sb[:, :, :],
                                   psE.rearrange("p (a b) -> p a b", a=2))
                    scr = em_sb.tile([64, 2, 128], F32, tag="scr", name="scr")
                    nc.vector.tensor_tensor(out=scr, in0=emb_sb, in1=emb_sb,
                                            op=mybir.AluOpType.mult)
                    ss = em_sb.tile([64, 2], F32, tag="ss", name="ss")
                    nc.vector.reduce_sum(out=ss, in_=scr,
                                         axis=mybir.AxisListType.X)
                    nrm = em_sb.tile([64, 2], F32, tag="nrm", name="nrm")
                    nc.scalar.sqrt(nrm, ss)
                    nc.vector.tensor_scalar_max(nrm, nrm, 1e-12)
                    rinv = em_sb.tile([64, 2], F32, tag="rinv", name="rinv")
                    nc.vector.reciprocal(rinv, nrm)
                    embn = em_sb.tile([64, 2, 128], MMDT, tag="embn", name="embn")
                    rinv_b = _ap(rinv[:, :], 0,
                                 [list(rinv[:, :].ap[0]), [1, 2], [0, 128]])
                    nc.vector.tensor_tensor(out=embn, in0=emb_sb, in1=rinv_b,
                                            op=mybir.AluOpType.mult)
                    embns[f] = embn

                def emb_tr_part(f):
                    embn = embns.pop(f)
                    for sp in range(2):
                        pt = psR_p.tile([128, 64], MMDT, tag="ptr", name="ptr")
                        nc.tensor.transpose(pt, embn[:, sp, :],
                                            ident[0:64, 0:64])
                        dst = _ap(embnT[:, 0:1], f * 128 + sp,
                                  [list(embnT[:, 0:1].ap[0]), [2, 64]])
                        nc.scalar.copy(dst, pt)

                pend = None
                pend_tr = None
                for c in range(NCHUNK):
                    rhs = cv_sb.tile([36, 16 * 256], MMDT, tag="rhs",
                                     name="rhs")
                    nc.sync.dma_start(out=rhs, in_=rx[c, :, :])
                    h1c = cv_h1.tile([64, NPAIR, PADZ], MMDT, tag="h1c",
                                     name="h1c")
                    ppitch = list(h1c[:, :, :].ap[0])
                    conv1_sub(c, rhs, h1c, ppitch, 0)
                    if pend_tr is not None:
                        emb_tr_part(pend_tr)
                        pend_tr = None
                    if pend is not None:
                        conv2_half(c - 1, pend, 0)
                    conv1_sub(c, rhs, h1c, ppitch, 1)
                    if pend is not None:
                        conv2_half(c - 1, pend, 1)
                        if c >= 3 and (c - 3) % 2 == 1:
                            f = (c - 3) // 2
                            emb_mm_part(f)
                            pend_tr = f
                    pend = h1c
                conv2_half(NCHUNK - 1, pend, 0)
                conv2_half(NCHUNK - 1, pend, 1)
                if pend_tr is not None:
                    emb_tr_part(pend_tr)
                for f in (T - 3, T - 2, T - 1):
                    emb_mm_part(f)
                    emb_tr_part(f)

            # -------- affinities, softmax, chains, loss --------
            with tc.tile_pool(name="aw_sb", bufs=1) as aw_sb, \
                 tc.tile_pool(name="at_sb", bufs=2) as at_sb, \
                 tc.tile_pool(name="aw_ps", bufs=4, space="PSUM") as aw_ps:
                aa = aw_sb.tile([128, 18, 128], F32)
                for t in range(9):
                    psA = aw_ps.tile([128, 128], F32, tag="awp", name="psA")
                    nc.tensor.matmul(psA, embnT[:, t * 128:(t + 1) * 128],
                                     embnT[:, (t + 1) * 128:(t + 2) * 128],
                                     start=True, stop=True)
                    nc.scalar.mul(aa[:, t, :], psA, 1.0 / TAU)
                    nc.sync.dma_start(out=a_out[t, :, :], in_=aa[:, t, :])
                    psB = aw_ps.tile([128, 128], F32, tag="awp", name="psB")
                    nc.tensor.matmul(psB, embnT[:, (t + 1) * 128:(t + 2) * 128],
                                     embnT[:, t * 128:(t + 1) * 128],
                                     start=True, stop=True)
                    nc.scalar.mul(aa[:, 17 - t, :], psB, 1.0 / TAU)

                # batched row-softmax, two halves (t 9..17 first so the
                # chain-feeding transposes can start while half 2 runs)
                s_sb = aw_sb.tile([128, 18, 128], MMDT)
                need = list(range(1, 8)) + list(range(10, 18))
                st_idx = {t: i for i, t in enumerate(need)}
                st_sb = aw_sb.tile([128, 15, 128], MMDT)
                halves = [(9, 18), (0, 9)]
                for lo, hi in halves:
                    n = hi - lo
                    negmx = aw_sb.tile([128, n], F32, tag="negmx", name="negmx")
                    nc.vector.reduce_max(out=negmx, in_=aa[:, lo:hi, :],
                                         axis=mybir.AxisListType.X, negate=True)
                    negmx_b = _ap(negmx[:, :], 0,
                                  [list(negmx[:, :].ap[0]), [1, n], [0, 128]])
                    ex = aw_sb.tile([128, n, 128], F32, tag="ex", name="ex")
                    nc.vector.tensor_tensor(out=ex, in0=aa[:, lo:hi, :],
                                            in1=negmx_b,
                                            op=mybir.AluOpType.add)
                    nc.scalar.activation(out=ex, in_=ex,
                                         func=mybir.ActivationFunctionType.Exp)
                    sme = aw_sb.tile([128, n], F32, tag="sme", name="sme")
                    nc.vector.reduce_sum(out=sme, in_=ex,
                                         axis=mybir.AxisListType.X)
                    rsx = aw_sb.tile([128, n], F32, tag="rsx", name="rsx")
                    nc.vector.reciprocal(rsx, sme)
                    rs_b = _ap(rsx[:, :], 0,
                               [list(rsx[:, :].ap[0]), [1, n], [0, 128]])
                    nc.vector.tensor_tensor(out=s_sb[:, lo:hi, :], in0=ex,
                                            in1=rs_b,
                                            op=mybir.AluOpType.mult)
                    for t in need:
                        if lo <= t < hi:
                            pst = aw_ps.tile([128, 128], MMDT, tag="awp",
                                             name="pst")
                            nc.tensor.transpose(pst, s_sb[:, t, :], ident)
                            nc.scalar.copy(st_sb[:, st_idx[t], :], pst)

                # 8 chain products interleaved round-robin
                att8 = aw_sb.tile([128, 8, 128], F32)
                chains = {}
                for k in range(1, 9):
                    seq = (list(range(k)) + list(range(18 - k, 18)))[1:]
                    chains[k] = (seq, 1, s_sb[:, seq[0], :])
                copy_eng = 0
                while chains:
                    for k in sorted(chains):
                        seq, pos, cur = chains[k]
                        if pos >= len(seq):
                            psT = aw_ps.tile([128, 128], MMDT, tag="awp",
                                             name="psT")
                            nc.tensor.transpose(psT, cur, ident)
                            nc.scalar.copy(att8[:, k - 1, :], psT)
                            del chains[k]
                            continue
                        psC = aw_ps.tile([128, 128], F32, tag="psC", bufs=4,
                                         name="psC")
                        nc.tensor.matmul(psC, st_sb[:, st_idx[seq[pos]], :],
                                         cur, start=True, stop=True)
                        nxt = at_sb.tile([128, 128], MMDT, tag=f"at{k % 4}",
                                         name="at")
                        if copy_eng == 0:
                            nc.vector.tensor_copy(nxt, psC)
                        else:
                            nc.scalar.copy(nxt, psC)
                        copy_eng ^= 1
                        chains[k] = (seq, pos + 1, nxt[:, :])

                # batched column log-softmax diagonals for all 8 k's
                negm8 = aw_sb.tile([128, 8], F32)
                nc.vector.reduce_max(out=negm8, in_=att8,
                                     axis=mybir.AxisListType.X, negate=True)
                negm8_b = _ap(negm8[:, :], 0,
                              [list(negm8[:, :].ap[0]), [1, 8], [0, 128]])
                ex8 = aw_sb.tile([128, 8, 128], F32)
                nc.vector.tensor_tensor(out=ex8, in0=att8, in1=negm8_b,
                                        op=mybir.AluOpType.add)
                nc.scalar.activation(out=ex8, in_=ex8,
                                     func=mybir.ActivationFunctionType.Exp)
                se8 = aw_sb.tile([128, 8], F32)
                nc.vector.reduce_sum(out=se8, in_=ex8, axis=mybir.AxisListType.X)
                ln8 = aw_sb.tile([128, 8], F32)
                nc.scalar.activation(out=ln8, in_=se8,
                                     func=mybir.ActivationFunctionType.Ln)
                lse8 = aw_sb.tile([128, 8], F32)
                nc.vector.tensor_tensor(out=lse8, in0=ln8, in1=negm8,
                                        op=mybir.AluOpType.subtract)
                ident_b = _ap(identf[:, :], 0,
                              [list(identf[:, :].ap[0]), [0, 8], [1, 128]])
                msk = aw_sb.tile([128, 8, 128], F32)
                nc.vector.tensor_tensor(out=msk, in0=att8, in1=ident_b,
                                        op=mybir.AluOpType.mult)
                dg8 = aw_sb.tile([128, 8], F32)
                nc.vector.reduce_sum(out=dg8, in_=msk, axis=mybir.AxisListType.X)
                ctr = aw_sb.tile([128, 8], F32)
                nc.vector.tensor_tensor(out=ctr, in0=lse8, in1=dg8,
                                        op=mybir.AluOpType.subtract)
                lacc = aw_sb.tile([128, 1], F32)
                nc.vector.reduce_sum(out=lacc, in_=ctr, axis=mybir.AxisListType.X)
                nc.sync.dma_start(out=lossp[:, :], in_=lacc)
    nc.compile()
    return nc


_NC = None


def _get_nc():
    global _NC
    if _NC is None:
        _NC = _build()
    return _NC


def _host_pack(seq, conv1_w, conv1_b, conv2_w, conv2_b, lin_w, lin_b):
    npdt = mybir.dt.np(MMDT)
    x = seq.reshape(B, NIMG, H, W).astype(np.float32)
    xp = np.pad(x, ((0, 0), (0, 0), (0, 2), (0, 2)))
    P = np.empty((B, NIMG, 9, 16, 16), np.float32)
    for dy in range(3):
        for dx in range(3):
            P[:, :, dy * 3 + dx] = xp[:, :, dy:dy + 32:2, dx:dx + 32:2]
    P2 = P.reshape(B, NCHUNK, 16, 4, 9, 256)
    R = np.ascontiguousarray(P2.transpose(0, 1, 3, 4, 2, 5)).reshape(
        B, NCHUNK, 36, 16 * 256).astype(npdt)

    w1p = np.zeros((36, 128), np.float32)
    for s in range(4):
        w1p[s * 9:(s + 1) * 9, s * 32:(s + 1) * 32] = conv1_w.reshape(32, 9).T
    w2p = np.zeros((64, 9, 128), np.float32)
    for sp in range(2):
        w2p[sp * 32:(sp + 1) * 32, :, sp * 64:(sp + 1) * 64] = \
            conv2_w.reshape(64, 32, 9).transpose(1, 2, 0)
    lin2 = np.vstack([lin_w.astype(np.float32) / 64.0,
                      lin_w.astype(np.float32) / 64.0])
    linb2 = np.tile(lin_b.astype(np.float32)[None, :], (128, 1))
    c1b = np.tile(conv1_b.astype(np.float32), 4)[:, None]
    c2b = np.tile(conv2_b.astype(np.float32), 2)[:, None]
    ident = np.eye(128, dtype=np.float32)
    shared = {
        "w1p": w1p.astype(npdt), "w2p": w2p.astype(npdt),
        "lin": lin2.astype(npdt), "linb": linb2.astype(npdt),
        "c1b": c1b, "c2b": c2b,
        "onesr": np.ones((128, 64), npdt),
        "identr": ident.astype(npdt), "identf_in": ident,
    }
    return [{"rx": np.ascontiguousarray(R[b]), **shared} for b in range(B)]


def kernel(seq, conv1_w, conv1_b, conv2_w, conv2_b, lin_w, lin_b):
    seq, conv1_w, conv1_b, conv2_w, conv2_b, lin_w, lin_b = (
        np.asarray(a) for a in
        (seq, conv1_w, conv1_b, conv2_w, conv2_b, lin_w, lin_b))
    nc = _get_nc()
    in_maps = _host_pack(seq, conv1_w, conv1_b, conv2_w, conv2_b,
                         lin_w, lin_b)
    res = run_bass_kernel_spmd(nc, in_maps, list(range(B)))
    A = np.stack([res.results[b]["A_out"] for b in range(B)]).astype(np.float32)
    tot = sum(float(res.results[b]["lossp"].sum()) for b in range(B))
    loss = np.float32(tot / (B * N * N))
    return (loss, A)
